# revision 4
# baseline (speedup 1.0000x reference)
"""nn_KFactor vq_codebook kernel for 8 TRN2 NeuronCores.

Math: label[b] = argmin_n ||x_b - P_n x_b||^2 = argmax_n q[n,b],
q[n,b] = ||U_n^T x_b||^2 where U_n = D_n (D_n^T D_n)^{-1/2} (orthonormal
basis of span(D_n), polar factor).  se_loss = sum_b (||x_b||^2 - max_n q).

Device per core (B-shard of 2048 samples):
  mm1 (fp32): s = [U_2p | U_2p+1]^T x^T     -> psum [128, 512] per (pair, chunk)
  ACT:        s2 = Square(s)                -> sbuf
  mm2 (fp32): q += ones_p^T s2 (accumulate over 33 pairs) -> psum [66, 512]
  (pair 32 = identity -> rows 64,65 give ||x||^2 halves)
  DVE 32x32 transposes -> qT [128b, 64n]; max_with_indices -> label, qmax.
Host: concat labels, se = sum(xx) - sum(qmax).
"""
import sys
sys.path.insert(0, "/opt/trn_rl_repo")
import numpy as np
from contextlib import ExitStack

import concourse.bass as bass
import concourse.tile as tile
from concourse import bacc, mybir
from concourse.bass_utils import run_bass_kernel_spmd

B, N, F, K = 16384, 64, 128, 64
NCORES = 8
BS = B // NCORES          # 2048
NP_ = 33                  # 32 cluster pairs + 1 identity pair
CH = 4                    # chunks of 512 samples
CB = 512
F32 = mybir.dt.float32

_cache = {}


def _build():
    nc = bacc.Bacc()
    xt_d = nc.declare_dram_parameter("xt", [F, BS], F32, isOutput=False)
    w_d = nc.declare_dram_parameter("w", [NP_, F, 128], F32, isOutput=False)
    on_d = nc.declare_dram_parameter("ones", [NP_, F, 66], F32, isOutput=False)
    lab_d = nc.declare_dram_parameter("label", [16, 128], mybir.dt.int32, isOutput=True)
    qm_d = nc.declare_dram_parameter("qmaxsum", [128, 1], F32, isOutput=True)
    xx_d = nc.declare_dram_parameter("xxsum", [2, 1], F32, isOutput=True)

    with tile.TileContext(nc) as tc, ExitStack() as ctx:
        const = ctx.enter_context(tc.tile_pool(name="const", bufs=1))
        sb = ctx.enter_context(tc.tile_pool(name="sb", bufs=3))
        ps = ctx.enter_context(tc.tile_pool(name="ps", bufs=3, space="PSUM"))
        psq = ctx.enter_context(tc.tile_pool(name="psq", bufs=2, space="PSUM"))

        xt = const.tile([F, BS], F32, tag="xt")
        nc.sync.dma_start(xt[:], xt_d[:])
        wts = []
        ons = []
        for p in range(NP_):
            wt = const.tile([F, 128], F32, tag=f"w{p}")
            nc.sync.dma_start(wt[:], w_d[p])
            wts.append(wt)
            ot = const.tile([F, 66], F32, tag=f"o{p}")
            nc.sync.dma_start(ot[:], on_d[p])
            ons.append(ot)
        q_sb = const.tile([66, BS], F32, tag="qsb")

        for c in range(CH):
            qp = psq.tile([66, CB], F32, tag="qp")
            for p in range(NP_):
                sp = ps.tile([128, CB], F32, tag="s")
                nc.tensor.matmul(sp[:], wts[p][:], xt[:, bass.ts(c, CB)],
                                 start=True, stop=True)
                s2 = sb.tile([128, CB], F32, tag="s2")
                nc.scalar.activation(s2[:], sp[:],
                                     mybir.ActivationFunctionType.Square)
                nc.tensor.matmul(qp[:], ons[p][:], s2[:],
                                 start=(p == 0), stop=(p == NP_ - 1))
            nc.vector.tensor_copy(q_sb[:, bass.ts(c, CB)], qp[:])

        # sum over samples of xx halves (rows 64,65)
        xs = const.tile([2, 1], F32, tag="xs")
        nc.vector.tensor_reduce(xs[:], q_sb[64:66, :], axis=mybir.AxisListType.X, op=mybir.AluOpType.add)
        nc.sync.dma_start(xx_d[:], xs[:])

        qacc = const.tile([128, 1], F32, tag="qacc")
        for t in range(16):
            qT = sb.tile([128, 64], F32, tag="qT")
            for i2 in range(4):
                for j2 in range(2):
                    nc.vector.transpose(
                        qT[32 * i2:32 * i2 + 32, 32 * j2:32 * j2 + 32],
                        q_sb[32 * j2:32 * j2 + 32,
                             128 * t + 32 * i2:128 * t + 32 * i2 + 32])
            mx = sb.tile([128, 8], F32, tag="mx")
            ix = sb.tile([128, 8], mybir.dt.uint32, tag="ix")
            nc.vector.max(mx[:], qT[:])
            nc.vector.max_index(ix[:], mx[:], qT[:])
            lab = sb.tile([128, 1], mybir.dt.int32, tag="lab")
            nc.vector.tensor_copy(lab[:], ix[:, 0:1])
            nc.sync.dma_start(lab_d[t], lab[:, 0])
            if t == 0:
                nc.vector.tensor_copy(qacc[:], mx[:, 0:1])
            else:
                nc.vector.tensor_add(qacc[:], qacc[:], mx[:, 0:1])
        nc.sync.dma_start(qm_d[:], qacc[:])
    nc.finalize()
    return nc


def _host_prep(x, D):
    # polar factors U_n = D_n G_n^{-1/2}; ~0.3% of total FLOPs, host-side
    G = np.einsum('nij,nik->njk', D.astype(np.float64), D.astype(np.float64))
    evals, evecs = np.linalg.eigh(G)
    Gmh = np.einsum('nij,nj,nkj->nik', evecs, 1.0 / np.sqrt(evals), evecs)
    U = np.einsum('nij,njk->nik', D.astype(np.float64), Gmh).astype(np.float32)
    W = np.zeros((NP_, F, 128), np.float32)
    for p in range(32):
        W[p, :, 0:64] = U[2 * p]
        W[p, :, 64:128] = U[2 * p + 1]
    W[32] = np.eye(128, dtype=np.float32)
    ONES = np.zeros((NP_, F, 66), np.float32)
    for p in range(NP_):
        ONES[p, 0:64, 2 * p] = 1.0
        ONES[p, 64:128, 2 * p + 1] = 1.0
    return W, ONES


def kernel(x, D):
    x = np.asarray(x, np.float32)
    D = np.asarray(D, np.float32)
    if "nc" not in _cache:
        _cache["nc"] = _build()
    nc = _cache["nc"]
    W, ONES = _host_prep(x, D)
    in_maps = []
    for c in range(NCORES):
        xs = np.ascontiguousarray(x[c * BS:(c + 1) * BS].T)
        in_maps.append({"xt": xs, "w": W, "ones": ONES})
    res = run_bass_kernel_spmd(nc, in_maps, list(range(NCORES)))
    labels = []
    se = 0.0
    for c in range(NCORES):
        r = res.results[c]
        labels.append(r["label"].reshape(-1).astype(np.int32))
        se += float(r["xxsum"].sum()) - float(r["qmaxsum"].sum())
    label = np.concatenate(labels)
    return np.float32(se), label
